# revision 56
# baseline (speedup 1.0000x reference)
"""Trainium2 Bass kernel for nn_BaseAttention (B=2, N=2048, E=2048, H=16, D=128).

Sharding: 8 cores; core c handles batch b=c//4, head-group hg=c%4 (4 heads).
Each core computes q/k/v projections for its heads, causal flash-style
attention, and a partial out-projection (contraction over its 512 head dims).
Host sums the 4 partial outputs per batch (tensor-parallel unshard).

The projections and the out-projection run as fp8e4 DoubleRow matmuls
(2 contraction k-tiles per instruction at 0.5 cycles/row): each operand is
split host-side (or on-device for the attention output) into an e4m3 hi part
plus an e4m3 lo residual, and three accumulation passes (hi*hi, hi*lo, lo*hi)
reconstruct the product to ~bf16 accuracy. Power-of-2 per-tensor scales keep
the fp8 mantissa in range; the descale folds into the psum-eviction copy.
QK^T and A@V stay fp16 (their contraction chains are too short for DoubleRow
to pay off).

Schedule: attention chunk 0 is emitted between projection chunks 1 and 2 so
its exp latency hides under projection matmuls; out-projection chains for
chunk ci-1 are interleaved one-per-QK-pair into chunk ci as PE filler while
ScalarE works through exp. Diagonal score tiles only compute the causally
valid column range. exp runs on ScalarE straight out of PSUM; softmax
denominators use a 0.25-column in the A@V matmul so the normalization also
applies the 4x fp8 pre-scale of the attention output. Attention output is
transposed on the PE via an identity matrix (chunks 1-3) and fp8-split by
DVE straight out of PSUM; triangle masks run on Pool (GPSIMD is SBUF-only).
"""

import math
import sys
import time

sys.path.insert(0, "/opt/trn_rl_repo")

import numpy as np
import ml_dtypes

import concourse.bass as bass
import concourse.mybir as mybir
import concourse.tile as tile
from concourse import bacc
from concourse.bass_utils import run_bass_kernel_spmd

B, N, E, H = 2, 2048, 2048, 16
D = E // H            # 128
HPC = 4               # heads per core
DC = HPC * D          # 512 head dims per core
NCORES = 8
P = 128
NCH = N // 512        # 4 n-chunks of 512
ET = E // P           # 16 e-tiles of 128
SA = 4.0              # fp8 pre-scale of the attention output

F32 = mybir.dt.float32
BF16 = mybir.dt.bfloat16
FP16 = mybir.dt.float16
FP8 = mybir.dt.float8e4
E4M3 = ml_dtypes.float8_e4m3
DR = mybir.MatmulPerfMode.DoubleRow


def build_nc(sx, sq, sk, sv, so):
    nc = bacc.Bacc("TRN2", target_bir_lowering=False, debug=False,
                   num_devices=NCORES)

    xTh = nc.dram_tensor("xTh", [E, N], FP8, kind="ExternalInput")
    xTl = nc.dram_tensor("xTl", [E, N], FP8, kind="ExternalInput")
    wqTh = nc.dram_tensor("wqTh", [E, DC], FP8, kind="ExternalInput")
    wqTl = nc.dram_tensor("wqTl", [E, DC], FP8, kind="ExternalInput")
    wkTh = nc.dram_tensor("wkTh", [E, DC], FP8, kind="ExternalInput")
    wkTl = nc.dram_tensor("wkTl", [E, DC], FP8, kind="ExternalInput")
    wvTh = nc.dram_tensor("wvTh", [E, DC], FP8, kind="ExternalInput")
    wvTl = nc.dram_tensor("wvTl", [E, DC], FP8, kind="ExternalInput")
    woTh = nc.dram_tensor("woTh", [DC, E], FP8, kind="ExternalInput")
    woTl = nc.dram_tensor("woTl", [DC, E], FP8, kind="ExternalInput")
    maskin = nc.dram_tensor("maskin", [P, 4, 512], FP16, kind="ExternalInput")
    ident = nc.dram_tensor("ident", [P, P], FP16, kind="ExternalInput")
    out = nc.dram_tensor("out", [N, E], FP16, kind="ExternalOutput")

    iq = 1.0 / (sx * sq)       # q psum descale
    ik = 1.0 / (sx * sk)
    iv = 1.0 / (sx * sv)
    io = 1.0 / (SA * so)

    xTh_r = xTh.ap().rearrange("(eo p) n -> p eo n", p=P)      # [128,16,2048]
    xTl_r = xTl.ap().rearrange("(eo p) n -> p eo n", p=P)
    wqTh_r = wqTh.ap().rearrange("(eo p) d -> p eo d", p=P)    # [128,16,512]
    wqTl_r = wqTl.ap().rearrange("(eo p) d -> p eo d", p=P)
    wkTh_r = wkTh.ap().rearrange("(eo p) d -> p eo d", p=P)
    wkTl_r = wkTl.ap().rearrange("(eo p) d -> p eo d", p=P)
    wvTh_r = wvTh.ap().rearrange("(eo p) d -> p eo d", p=P)
    wvTl_r = wvTl.ap().rearrange("(eo p) d -> p eo d", p=P)
    woTh_r = woTh.ap().rearrange("(t p) e -> p t e", p=P)      # [128,4,2048]
    woTl_r = woTl.ap().rearrange("(t p) e -> p t e", p=P)

    with tile.TileContext(nc) as tc:
        # ---------------- constants + spill tensors ----------------
        consts = tc.alloc_tile_pool(name="consts", bufs=1)
        _longlived = [consts]
        mask_sb = consts.tile([P, 4, 512], FP16)
        ident_sb = consts.tile([P, P], FP16)
        # prefire the Exp table load so it overlaps the input DMA head
        dummy = consts.tile([1, 8], F32)
        nc.vector.memset(dummy, 0.0)
        nc.scalar.activation(out=dummy, in_=dummy,
                             func=mybir.ActivationFunctionType.Exp)

        dram = tc.alloc_tile_pool(name="dram", bufs=1, space="DRAM")
        _longlived.append(dram)
        attd = dram.tile([HPC, N, D], FP16)          # normalized attn out

        # per-core activations, SBUF-resident across the whole kernel
        big = tc.alloc_tile_pool(name="big", bufs=1)
        _longlived.append(big)
        qs = big.tile([P, HPC, N], FP16)                  # q^T, heads stacked
        ks = big.tile([P, HPC, N], FP16)                  # k^T
        v_all = big.tile([P, N // P, HPC, D + 4], FP16)   # [V | 1/SA] per blk
        # 0.25-column: A@V col D becomes denom/SA, so the normalization
        # multiply bakes in the fp8 pre-scale SA of the attention output
        nc.vector.memset(v_all[:, :, :, D:D + 1], 1.0 / SA)

        outT16 = big.tile([P, HPC, N], FP16)    # SA*att^T
        outTh = big.tile([P, HPC, N], FP8)      # fp8 hi
        outTl = big.tile([P, HPC, N], FP8)      # fp8 lo residual
        woh_sb = big.tile([P, HPC, E], FP8)
        wol_sb = big.tile([P, HPC, E], FP8)

        # attention pools spanning the proj phase (chunk 0 runs inside it)
        att_pool = tc.alloc_tile_pool(name="att_pool", bufs=2)
        _longlived.append(att_pool)
        rs_pool = tc.alloc_tile_pool(name="rs_pool", bufs=8)
        _longlived.append(rs_pool)
        qk_ps = tc.alloc_tile_pool(name="qk_ps", bufs=2, space="PSUM")
        _longlived.append(qk_ps)
        av_ps = tc.alloc_tile_pool(name="av_ps", bufs=2, space="PSUM")
        _longlived.append(av_ps)

        # out-projection chain state (filled after each attention chunk,
        # drained one chain per QK pair as PE filler). "fresh" chains wait
        # one full window before becoming fillable so the PE never stalls
        # on the producing head's quantize pipeline.
        op_state = {"pool": None, "ev": None, "pending": [], "fresh": [],
                    "ostage": {}, "tr": None, "drain": False}

        def emit_opchain():
            nb, ec = op_state["pending"].pop(0)
            nbsl = slice(nb * P, (nb + 1) * P)
            esl = slice(ec * 512, (ec + 1) * 512)
            ps = op_state["pool"].tile([P, 512], F32, tag="opps",
                                       name=f"opps_{nb}_{ec}")
            n_mm = 0
            for apass, wpass in ((outTh, woh_sb), (outTh, wol_sb),
                                 (outTl, woh_sb)):
                for tp in range(HPC // 2):
                    tsl = slice(2 * tp, 2 * tp + 2)
                    nc.tensor.matmul(
                        ps,
                        lhsT=apass[:, tsl, nbsl],
                        rhs=wpass[:, tsl, esl],
                        start=(n_mm == 0), stop=(n_mm == 5),
                        perf_mode=DR,
                    )
                    n_mm += 1
            if ec % 2 == 0:
                ost = op_state["ev"].tile([P, 2, 512], FP16, tag="opev",
                                          name=f"ost_{nb}_{ec}")
                op_state["ostage"][nb] = ost
            else:
                ost = op_state["ostage"][nb]
            if op_state["drain"] and ec % 2 == 1:
                nc.scalar.mul(ost[:, ec % 2, :], ps, io)
            else:
                nc.vector.tensor_scalar_mul(
                    out=ost[:, ec % 2, :], in0=ps, scalar1=io)
            if ec % 2 == 1:
                eng = nc.scalar if (op_state["drain"] and ec == 1) else nc.sync
                eng.dma_start(
                    out=out.ap()[nbsl, (ec - 1) * 512:(ec + 1) * 512],
                    in_=ost)

        pt_tiles = {}

        def qk_unit(ci, h, pt_pool, pt_tag, fill_cap=0):
            """Score tiles [j_block, i_chunk] + exp for one (chunk, head).

            Diagonal blocks (last 4) only compute the causally valid column
            range; exp runs full-width over the pair (the invalid regions
            hold stale-but-finite psum and are never read by A@V). Out-proj
            chains interleave one-per-pair as PE filler up to fill_cap.
            """
            BJ = 4 * (ci + 1)
            if h >= 1 and op_state["fresh"]:
                op_state["pending"].extend(op_state["fresh"])
                op_state["fresh"] = []
            pt = pt_pool.tile([P, BJ, 512], FP16, tag=pt_tag,
                              name=f"pt_{ci}_{h}")
            pt_tiles[(ci, h)] = pt
            filled = 0
            for bjp in range(BJ // 2):
                ps = qk_ps.tile([P, 2, 512], F32, tag="qkps",
                                name=f"qkps_{ci}_{h}_{bjp}")
                for u in range(2):
                    bj = 2 * bjp + u
                    lo = max(0, (bj - (BJ - 4)) * P)
                    nc.tensor.matmul(
                        ps[:, u, lo:],
                        lhsT=ks[:, h, bj * P:(bj + 1) * P],
                        rhs=qs[:, h, ci * 512 + lo:(ci + 1) * 512],
                        start=True, stop=True,
                    )
                nc.scalar.activation(
                    out=pt[:, 2 * bjp:2 * bjp + 2, :], in_=ps,
                    func=mybir.ActivationFunctionType.Exp)
                if filled < fill_cap and op_state["pending"]:
                    emit_opchain()
                    filled += 1

        def av_unit(ci, h):
            """Strip masks + A@V + normalize + spill/transpose/fp8-split."""
            BJ = 4 * (ci + 1)
            pt = pt_tiles.pop((ci, h))
            # triangle masks: only the [rr*128, rr*128+128) strip of each
            # diagonal block is read by a chain that needs masking
            for rr in range(4):
                bj = BJ - 4 + rr
                csl = slice(rr * P, (rr + 1) * P)
                nc.gpsimd.tensor_mul(
                    out=pt[:, bj, csl], in0=pt[:, bj, csl],
                    in1=mask_sb[:, rr, csl])

            # A @ [V | 1/SA]: out rows are queries, col 128 holds denom/SA;
            # normalize (and fp8-prescale) on eviction. Chain ib only sums
            # blocks bj <= BJ-4+ib — later diagonal blocks are all-masked.
            att_h = att_pool.tile([P, 4, D], FP16, tag="atth",
                                  name=f"att_{ci}_{h}")
            for ib in range(4):
                nbj = BJ - 3 + ib
                avp = av_ps.tile([P, D + 4], F32, tag="avps",
                                 name=f"avps_{ci}_{h}_{ib}")
                isl = slice(ib * P, (ib + 1) * P)
                for bj in range(nbj):
                    nc.tensor.matmul(
                        avp[:, :D + 1],
                        lhsT=pt[:, bj, isl],
                        rhs=v_all[:, bj, h, :D + 1],
                        start=(bj == 0), stop=(bj == nbj - 1),
                    )
                rs = rs_pool.tile([P, 1], F32, tag="rs",
                                  name=f"rs_{ci}_{h}_{ib}")
                nc.vector.reciprocal_approx_fast(
                    out=rs, in_=avp[:, D:D + 1])
                nc.vector.tensor_scalar_mul(
                    out=att_h[:, ib, :], in0=avp[:, :D], scalar1=rs)

            # transpose + fp8 hi/lo split right away so the out-projection
            # chains for this chunk unlock early. Chunk 0 (inside the proj
            # phase, PSUM full) takes the DRAM spill + XBAR-transpose path;
            # later chunks transpose on the PE via the identity matrix and
            # quantize straight out of PSUM.
            nsl = slice(ci * 512, (ci + 1) * 512)
            if op_state["tr"] is None:
                nc.sync.dma_start(
                    out=attd[h, nsl, :].rearrange("(io p) d -> p io d", p=P),
                    in_=att_h)
                nc.sync.dma_start_transpose(
                    out=outT16[:, h, nsl], in_=attd[h, nsl, :])
                nc.gpsimd.tensor_copy(out=outTh[:, h, nsl],
                                      in_=outT16[:, h, nsl])
                nc.vector.tensor_sub(
                    out=outTl[:, h, nsl], in0=outT16[:, h, nsl],
                    in1=outTh[:, h, nsl])
            else:
                for ib in range(4):
                    tr = av_ps.tile([P, P], FP16, tag="avps",
                                    name=f"tr_{ci}_{h}_{ib}")
                    nc.tensor.matmul(tr, lhsT=att_h[:, ib, :], rhs=ident_sb,
                                     is_transpose=True, start=True, stop=True)
                    asl = slice(ci * 512 + ib * P, ci * 512 + (ib + 1) * P)
                    nc.vector.tensor_copy(out=outTh[:, h, asl], in_=tr)
                    nc.vector.tensor_sub(
                        out=outTl[:, h, asl], in0=tr, in1=outTh[:, h, asl])
            if h == HPC - 1:
                # this chunk's out-projection blocks are now computable
                dest = "pending" if ci == 0 else "fresh"
                for nb in range(ci * 4, ci * 4 + 4):
                    for ec in range(NCH):
                        op_state[dest].append((nb, ec))

        # ---------------- phase 1: projections (+ attention chunk 0) -------
        pt0_pool = tc.alloc_tile_pool(name="pt0_pool", bufs=2)

        with (
            tc.tile_pool(name="wpool", bufs=1) as wpool,
            tc.tile_pool(name="xpool", bufs=2) as xpool,
            tc.tile_pool(name="pj_ps", bufs=2, space="PSUM") as pj_ps,
        ):
            wqh_sb = wpool.tile([P, ET, DC], FP8)
            wql_sb = wpool.tile([P, ET, DC], FP8)
            wkh_sb = wpool.tile([P, ET, DC], FP8)
            wkl_sb = wpool.tile([P, ET, DC], FP8)
            wvh_sb = wpool.tile([P, ET, DC], FP8)
            wvl_sb = wpool.tile([P, ET, DC], FP8)
            x_tiles = [None] * NCH

            def load_x(nch, eng):
                th = xpool.tile([P, ET, 512], FP8, tag="xh",
                                name=f"xh_sb{nch}")
                tl = xpool.tile([P, ET, 512], FP8, tag="xl",
                                name=f"xl_sb{nch}")
                nsl = slice(nch * 512, (nch + 1) * 512)
                eng.dma_start(out=th, in_=xTh_r[:, :, nsl])
                eng.dma_start(out=tl, in_=xTl_r[:, :, nsl])
                x_tiles[nch] = (th, tl)

            # preload split across the two HWDGE queues so each stream
            # arrives before its first consuming chain:
            #   SP queue:  wq (fine-grained head), wv, mask, x2
            #   Act queue: x0 (fine-grained), wk, x1, wo, x3
            x0h = xpool.tile([P, ET, 512], FP8, tag="xh", name="xh_sb0")
            x0l = xpool.tile([P, ET, 512], FP8, tag="xl", name="xl_sb0")
            x_tiles[0] = (x0h, x0l)
            HF = ET // 2
            for a, b in ((0, 2), (2, 4), (4, 8), (8, 16)):
                gs = slice(a, b)
                nc.sync.dma_start(out=wqh_sb[:, gs, :], in_=wqTh_r[:, gs, :])
                nc.scalar.dma_start(out=x0h[:, gs, :], in_=xTh_r[:, gs, 0:512])
            for gs in (slice(0, HF), slice(HF, ET)):
                nc.sync.dma_start(out=wql_sb[:, gs, :], in_=wqTl_r[:, gs, :])
                nc.scalar.dma_start(out=x0l[:, gs, :], in_=xTl_r[:, gs, 0:512])
            for gs in (slice(0, HF), slice(HF, ET)):
                nc.scalar.dma_start(out=wkh_sb[:, gs, :], in_=wkTh_r[:, gs, :])
            nc.scalar.dma_start(out=wkl_sb, in_=wkTl_r)
            # wv split across both queues so it lands before x1/wo transfers
            nc.sync.dma_start(out=wvh_sb[:, :HF, :], in_=wvTh_r[:, :HF, :])
            nc.scalar.dma_start(out=wvh_sb[:, HF:, :], in_=wvTh_r[:, HF:, :])
            nc.sync.dma_start(out=wvl_sb[:, :HF, :], in_=wvTl_r[:, :HF, :])
            nc.scalar.dma_start(out=wvl_sb[:, HF:, :], in_=wvTl_r[:, HF:, :])
            nc.sync.dma_start(out=mask_sb, in_=maskin.ap())
            nc.sync.dma_start(out=ident_sb, in_=ident.ap())
            load_x(1, nc.scalar)
            for t in range(HPC):
                nc.scalar.dma_start(out=woh_sb[:, t, :], in_=woTh_r[:, t, :])
                nc.scalar.dma_start(out=wol_sb[:, t, :], in_=woTl_r[:, t, :])

            def proj_qk(nch, which):
                xh_sb, xl_sb = x_tiles[nch]
                nsl = slice(nch * 512, (nch + 1) * 512)
                wh_sb, wl_sb, dst, dsc = {
                    "q": (wqh_sb, wql_sb, qs, iq),
                    "k": (wkh_sb, wkl_sb, ks, ik),
                }[which]
                # psum[dq_tile 128, 512 n] = sum_e W[e, dq] x[e, n]
                for t in range(HPC):
                    tsl = slice(t * P, (t + 1) * P)
                    ps = pj_ps.tile([P, 512], F32, tag="pjps",
                                    name=f"pjps_{which}{nch}_{t}")
                    n_mm = 0
                    for wpass, xpass in ((wh_sb, xh_sb), (wh_sb, xl_sb),
                                         (wl_sb, xh_sb)):
                        for kp in range(ET // 2):
                            es = slice(2 * kp, 2 * kp + 2)
                            nc.tensor.matmul(
                                ps,
                                lhsT=wpass[:, es, tsl],
                                rhs=xpass[:, es, :],
                                start=(n_mm == 0),
                                stop=(n_mm == 3 * ET // 2 - 1),
                                perf_mode=DR,
                            )
                            n_mm += 1
                    if nch == NCH - 1 and t < 2:
                        nc.scalar.mul(dst[:, t, nsl], ps, dsc)
                    else:
                        nc.vector.tensor_scalar_mul(
                            out=dst[:, t, nsl], in0=ps, scalar1=dsc)

            def proj_v(nch):
                xh_sb, xl_sb = x_tiles[nch]
                # psum[n_block 128, 512 dv] = sum_e x[e, n] Wv[e, dv]
                for nb in range(4):
                    nbsl = slice(nb * P, (nb + 1) * P)
                    ps = pj_ps.tile([P, 512], F32, tag="pjps",
                                    name=f"pjps_v{nch}_{nb}")
                    n_mm = 0
                    for xpass, wpass in ((xh_sb, wvh_sb), (xh_sb, wvl_sb),
                                         (xl_sb, wvh_sb)):
                        for kp in range(ET // 2):
                            es = slice(2 * kp, 2 * kp + 2)
                            nc.tensor.matmul(
                                ps,
                                lhsT=xpass[:, es, nbsl],
                                rhs=wpass[:, es, :],
                                start=(n_mm == 0),
                                stop=(n_mm == 3 * ET // 2 - 1),
                                perf_mode=DR,
                            )
                            n_mm += 1
                    ps_h = ps.rearrange("p (h d) -> p h d", h=HPC)
                    vdst = v_all[:, nch * 4 + nb, :, :D]
                    if nch == NCH - 1 and nb < 2:
                        nc.scalar.mul(vdst, ps_h, iv)
                    else:
                        nc.vector.tensor_scalar_mul(
                            out=vdst, in0=ps_h, scalar1=iv)

            def proj_chunk(nch):
                proj_qk(nch, "q")
                proj_qk(nch, "k")
                proj_v(nch)
                if nch + 2 < NCH:
                    load_x(nch + 2, nc.sync if nch == 0 else nc.scalar)

            proj_chunk(0)
            proj_chunk(1)
            # attention chunk 0 interleaves with proj chunks 2-3 at head
            # granularity: each head's exp latency hides under a group of
            # four projection chains before its A@V runs
            qk_unit(0, 0, pt0_pool, "pt0")
            proj_qk(2, "q")
            av_unit(0, 0)
            qk_unit(0, 1, pt0_pool, "pt0")
            proj_qk(2, "k")
            av_unit(0, 1)
            qk_unit(0, 2, pt0_pool, "pt0")
            proj_v(2)
            av_unit(0, 2)
            qk_unit(0, 3, pt0_pool, "pt0")
            proj_qk(3, "q")
            av_unit(0, 3)
            proj_qk(3, "k")
            proj_v(3)

        pt0_pool.release()

        # ---------------- phase 2: attention chunks 1-3 + out-projection ---
        op_ps = tc.alloc_tile_pool(name="op_ps", bufs=2, space="PSUM")
        op_ev = tc.alloc_tile_pool(name="op_ev", bufs=6)
        op_state["pool"] = op_ps
        op_state["ev"] = op_ev
        op_state["tr"] = True

        pt_pool = tc.alloc_tile_pool(name="pt_pool", bufs=2)
        fill_caps = {1: 3, 2: 4, 3: 5}
        for ci in range(1, NCH):
            for h in range(HPC):
                qk_unit(ci, h, pt_pool, f"pt{ci}", fill_cap=fill_caps[ci])
                av_unit(ci, h)

        # drain the remaining out-projection chains (last chunk's blocks)
        op_state["pending"].extend(op_state["fresh"])
        op_state["fresh"] = []
        op_state["drain"] = True
        while op_state["pending"]:
            emit_opchain()

        pt_pool.release()
        op_ev.release()
        op_ps.release()
        for _pl in reversed(_longlived):
            _pl.release()

    nc.compile()
    return nc


def _po2(target_rms, arr):
    rms = float(np.sqrt(np.mean(arr.astype(np.float64) ** 2)))
    if rms == 0.0 or not math.isfinite(rms):
        return 1.0
    return 2.0 ** round(math.log2(target_rms / rms))


def _hilo(a32):
    hi = a32.astype(E4M3)
    lo = (a32 - hi.astype(np.float32)).astype(E4M3)
    return hi, lo


def make_in_maps(x, Wq, Wkv, Wout):
    x = np.asarray(x, dtype=np.float32)
    Wq = np.asarray(Wq, dtype=np.float32)
    Wkv = np.asarray(Wkv, dtype=np.float32)
    Wout = np.asarray(Wout, dtype=np.float32)
    scale = np.float32(D ** -0.5)

    sx = _po2(3.0, x)
    sq = _po2(3.0, Wq * scale)
    sk = _po2(3.0, Wkv[:E])
    sv = _po2(3.0, Wkv[E:])
    so = _po2(3.0, Wout)
    scales = (sx, sq, sk, sv, so)

    # causal masks for the 4 diagonal offsets
    jj = np.arange(P)[:, None]
    ii = np.arange(512)[None, :]
    mask = np.zeros((P, 4, 512), dtype=np.float16)
    for rr in range(4):
        mask[:, rr, :] = (ii >= jj + rr * P).astype(np.float16)

    xT = [_hilo(np.ascontiguousarray(x[b].T) * sx) for b in range(B)]
    in_maps = []
    for c in range(NCORES):
        b, hg = divmod(c, 4)
        sl = slice(hg * DC, (hg + 1) * DC)
        wq_h, wq_l = _hilo(np.ascontiguousarray(Wq[sl, :].T) * (scale * sq))
        wk_h, wk_l = _hilo(np.ascontiguousarray(Wkv[sl, :].T) * sk)
        wv_h, wv_l = _hilo(
            np.ascontiguousarray(Wkv[E + sl.start:E + sl.stop, :].T) * sv)
        wo_h, wo_l = _hilo(np.ascontiguousarray(Wout[:, sl].T) * so)
        in_maps.append({
            "xTh": xT[b][0], "xTl": xT[b][1],
            "wqTh": wq_h, "wqTl": wq_l,
            "wkTh": wk_h, "wkTl": wk_l,
            "wvTh": wv_h, "wvTl": wv_l,
            "woTh": wo_h, "woTl": wo_l,
            "maskin": mask,
            "ident": np.eye(P, dtype=np.float16),
        })
    return in_maps, scales


_NC_CACHE = {}
_LAST_SCALES = [None]


def _get_nc(scales=None):
    if scales is None:
        scales = _LAST_SCALES[0]
        assert scales is not None, "call make_in_maps first"
    if scales not in _NC_CACHE:
        _NC_CACHE[scales] = build_nc(*scales)
    _LAST_SCALES[0] = scales
    return _NC_CACHE[scales]


def _run(in_maps, scales):
    nc = _get_nc(scales)
    return run_bass_kernel_spmd(nc, in_maps, core_ids=list(range(NCORES)))


def kernel(x, Wq, Wkv, Wout):
    in_maps, scales = make_in_maps(x, Wq, Wkv, Wout)
    res = _run(in_maps, scales)
    out = np.zeros((B, N, E), dtype=np.float32)
    for c in range(NCORES):
        out[c // 4] += res.results[c]["out"].astype(np.float32)
    return out


if __name__ == "__main__":
    t0 = time.time()
    rng = np.random.default_rng(0)
    ins = {
        "x": rng.standard_normal((B, N, E), dtype=np.float32),
        "Wq": rng.standard_normal((E, E), dtype=np.float32) * 0.02,
        "Wkv": rng.standard_normal((2 * E, E), dtype=np.float32) * 0.02,
        "Wout": rng.standard_normal((E, E), dtype=np.float32) * 0.02,
    }
    im, sc = make_in_maps(**ins)
    _get_nc(sc)
    print(f"build+compile: {time.time() - t0:.1f}s")


# revision 65
# speedup vs baseline: 1.0008x; 1.0008x over previous
"""Trainium2 Bass kernel for nn_BaseAttention (B=2, N=2048, E=2048, H=16, D=128).

Sharding: 8 cores; core c handles batch b=c//4, head-group hg=c%4 (4 heads).
Each core computes q/k/v projections for its heads, causal flash-style
attention, and a partial out-projection (contraction over its 512 head dims).
Host sums the 4 partial outputs per batch (tensor-parallel unshard).

The projections and the out-projection run as fp8e4 DoubleRow matmuls
(2 contraction k-tiles per instruction at 0.5 cycles/row): each operand is
split host-side (or on-device for the attention output) into an e4m3 hi part
plus an e4m3 lo residual, and three accumulation passes (hi*hi, hi*lo, lo*hi)
reconstruct the product to ~bf16 accuracy. Power-of-2 per-tensor scales keep
the fp8 mantissa in range; the descale folds into the psum-eviction copy.
QK^T and A@V stay fp16 (their contraction chains are too short for DoubleRow
to pay off).

Schedule: attention chunk 0 is emitted between projection chunks 1 and 2 so
its exp latency hides under projection matmuls; out-projection chains for
chunk ci-1 are interleaved one-per-QK-pair into chunk ci as PE filler while
ScalarE works through exp. Diagonal score tiles only compute the causally
valid column range. exp runs on ScalarE straight out of PSUM; softmax
denominators use a 0.25-column in the A@V matmul so the normalization also
applies the 4x fp8 pre-scale of the attention output. Attention output is
transposed on the PE via an identity matrix (chunks 1-3) and fp8-split by
DVE straight out of PSUM; triangle masks run on Pool (GPSIMD is SBUF-only).
"""

import math
import sys
import time

sys.path.insert(0, "/opt/trn_rl_repo")

import numpy as np
import ml_dtypes

import concourse.bass as bass
import concourse.mybir as mybir
import concourse.tile as tile
from concourse import bacc
from concourse.bass_utils import run_bass_kernel_spmd

B, N, E, H = 2, 2048, 2048, 16
D = E // H            # 128
HPC = 4               # heads per core
DC = HPC * D          # 512 head dims per core
NCORES = 8
P = 128
NCH = N // 512        # 4 n-chunks of 512
ET = E // P           # 16 e-tiles of 128
SA = 4.0              # fp8 pre-scale of the attention output

F32 = mybir.dt.float32
BF16 = mybir.dt.bfloat16
FP16 = mybir.dt.float16
FP8 = mybir.dt.float8e4
E4M3 = ml_dtypes.float8_e4m3
DR = mybir.MatmulPerfMode.DoubleRow


def build_nc(sx, sq, sk, sv, so):
    nc = bacc.Bacc("TRN2", target_bir_lowering=False, debug=False,
                   num_devices=NCORES)

    xTh = nc.dram_tensor("xTh", [E, N], FP8, kind="ExternalInput")
    xTl = nc.dram_tensor("xTl", [E, N], FP8, kind="ExternalInput")
    wqTh = nc.dram_tensor("wqTh", [E, DC], FP8, kind="ExternalInput")
    wqTl = nc.dram_tensor("wqTl", [E, DC], FP8, kind="ExternalInput")
    wkTh = nc.dram_tensor("wkTh", [E, DC], FP8, kind="ExternalInput")
    wkTl = nc.dram_tensor("wkTl", [E, DC], FP8, kind="ExternalInput")
    wvTh = nc.dram_tensor("wvTh", [E, DC], FP8, kind="ExternalInput")
    wvTl = nc.dram_tensor("wvTl", [E, DC], FP8, kind="ExternalInput")
    woTh = nc.dram_tensor("woTh", [DC, E], FP8, kind="ExternalInput")
    woTl = nc.dram_tensor("woTl", [DC, E], FP8, kind="ExternalInput")
    maskin = nc.dram_tensor("maskin", [P, 4, 512], FP16, kind="ExternalInput")
    ident = nc.dram_tensor("ident", [P, P], FP16, kind="ExternalInput")
    out = nc.dram_tensor("out", [N, E], FP16, kind="ExternalOutput")

    iq = 1.0 / (sx * sq)       # q psum descale
    ik = 1.0 / (sx * sk)
    iv = 1.0 / (sx * sv)
    io = 1.0 / (SA * so)

    xTh_r = xTh.ap().rearrange("(eo p) n -> p eo n", p=P)      # [128,16,2048]
    xTl_r = xTl.ap().rearrange("(eo p) n -> p eo n", p=P)
    wqTh_r = wqTh.ap().rearrange("(eo p) d -> p eo d", p=P)    # [128,16,512]
    wqTl_r = wqTl.ap().rearrange("(eo p) d -> p eo d", p=P)
    wkTh_r = wkTh.ap().rearrange("(eo p) d -> p eo d", p=P)
    wkTl_r = wkTl.ap().rearrange("(eo p) d -> p eo d", p=P)
    wvTh_r = wvTh.ap().rearrange("(eo p) d -> p eo d", p=P)
    wvTl_r = wvTl.ap().rearrange("(eo p) d -> p eo d", p=P)
    woTh_r = woTh.ap().rearrange("(t p) e -> p t e", p=P)      # [128,4,2048]
    woTl_r = woTl.ap().rearrange("(t p) e -> p t e", p=P)

    with tile.TileContext(nc) as tc:
        # ---------------- constants + spill tensors ----------------
        consts = tc.alloc_tile_pool(name="consts", bufs=1)
        _longlived = [consts]
        mask_sb = consts.tile([P, 4, 512], FP16)
        ident_sb = consts.tile([P, P], FP16)
        # prefire the Exp table load so it overlaps the input DMA head
        dummy = consts.tile([1, 8], F32)
        nc.vector.memset(dummy, 0.0)
        nc.scalar.activation(out=dummy, in_=dummy,
                             func=mybir.ActivationFunctionType.Exp)

        dram = tc.alloc_tile_pool(name="dram", bufs=1, space="DRAM")
        _longlived.append(dram)
        attd = dram.tile([HPC, N, D], FP16)          # normalized attn out

        # per-core activations, SBUF-resident across the whole kernel
        big = tc.alloc_tile_pool(name="big", bufs=1)
        _longlived.append(big)
        qs = big.tile([P, HPC, N], FP16)                  # q^T, heads stacked
        ks = big.tile([P, HPC, N], FP16)                  # k^T
        v_all = big.tile([P, N // P, HPC, D + 4], FP16)   # [V | 1/SA] per blk
        # 0.25-column: A@V col D becomes denom/SA, so the normalization
        # multiply bakes in the fp8 pre-scale SA of the attention output
        nc.vector.memset(v_all[:, :, :, D:D + 1], 1.0 / SA)

        outT16 = big.tile([P, HPC, N], FP16)    # SA*att^T
        outTh = big.tile([P, HPC, N], FP8)      # fp8 hi
        outTl = big.tile([P, HPC, N], FP8)      # fp8 lo residual
        woh_sb = big.tile([P, HPC, E], FP8)
        wol_sb = big.tile([P, HPC, E], FP8)

        # attention pools spanning the proj phase (chunk 0 runs inside it)
        att_pool = tc.alloc_tile_pool(name="att_pool", bufs=2)
        _longlived.append(att_pool)
        rs_pool = tc.alloc_tile_pool(name="rs_pool", bufs=8)
        _longlived.append(rs_pool)
        qk_ps = tc.alloc_tile_pool(name="qk_ps", bufs=2, space="PSUM")
        _longlived.append(qk_ps)
        av_ps = tc.alloc_tile_pool(name="av_ps", bufs=2, space="PSUM")
        _longlived.append(av_ps)

        # out-projection chain state (filled after each attention chunk,
        # drained one chain per QK pair as PE filler). "fresh" chains wait
        # one full window before becoming fillable so the PE never stalls
        # on the producing head's quantize pipeline.
        op_state = {"pool": None, "ev": None, "pending": [], "fresh": [],
                    "ostage": {}, "tr": None, "drain": False,
                    "drain_even": set()}

        def emit_opchain():
            nb, ec = op_state["pending"].pop(0)
            nbsl = slice(nb * P, (nb + 1) * P)
            esl = slice(ec * 512, (ec + 1) * 512)
            ps = op_state["pool"].tile([P, 512], F32, tag="opps",
                                       name=f"opps_{nb}_{ec}")
            n_mm = 0
            for apass, wpass in ((outTh, woh_sb), (outTh, wol_sb),
                                 (outTl, woh_sb)):
                for tp in range(HPC // 2):
                    tsl = slice(2 * tp, 2 * tp + 2)
                    nc.tensor.matmul(
                        ps,
                        lhsT=apass[:, tsl, nbsl],
                        rhs=wpass[:, tsl, esl],
                        start=(n_mm == 0), stop=(n_mm == 5),
                        perf_mode=DR,
                    )
                    n_mm += 1
            if ec % 2 == 0:
                ost = op_state["ev"].tile([P, 2, 512], FP16, tag="opev",
                                          name=f"ost_{nb}_{ec}")
                op_state["ostage"][nb] = ost
            else:
                ost = op_state["ostage"][nb]
            if op_state["drain"] and ec % 2 == 1:
                nc.scalar.mul(ost[:, ec % 2, :], ps, io)
            else:
                nc.vector.tensor_scalar_mul(
                    out=ost[:, ec % 2, :], in0=ps, scalar1=io)
            if op_state["drain"] and ec % 2 == 0:
                # drain: ship each half as soon as its eviction lands, on
                # alternating queues, to shorten the final DMA pipeline
                nc.scalar.dma_start(
                    out=out.ap()[nbsl, ec * 512:(ec + 1) * 512],
                    in_=ost[:, 0, :])
                op_state["drain_even"].add((nb, ec))
            elif ec % 2 == 1:
                if op_state["drain"] and (nb, ec - 1) in op_state["drain_even"]:
                    nc.sync.dma_start(
                        out=out.ap()[nbsl, ec * 512:(ec + 1) * 512],
                        in_=ost[:, 1, :])
                else:
                    # pair DMA: the even half was evicted (and not shipped)
                    # before the drain began
                    nc.sync.dma_start(
                        out=out.ap()[nbsl, (ec - 1) * 512:(ec + 1) * 512],
                        in_=ost)

        pt_tiles = {}

        def qk_unit(ci, h, pt_pool, pt_tag, fill_cap=0):
            """Score tiles [j_block, i_chunk] + exp for one (chunk, head).

            Diagonal blocks (last 4) only compute the causally valid column
            range; exp runs full-width over the pair (the invalid regions
            hold stale-but-finite psum and are never read by A@V). Out-proj
            chains interleave one-per-pair as PE filler up to fill_cap.
            """
            BJ = 4 * (ci + 1)
            if h >= 1 and op_state["fresh"]:
                op_state["pending"].extend(op_state["fresh"])
                op_state["fresh"] = []
            pt = pt_pool.tile([P, BJ, 512], FP16, tag=pt_tag,
                              name=f"pt_{ci}_{h}")
            pt_tiles[(ci, h)] = pt
            filled = 0
            for bjp in range(BJ // 2):
                ps = qk_ps.tile([P, 2, 512], F32, tag="qkps",
                                name=f"qkps_{ci}_{h}_{bjp}")
                for u in range(2):
                    bj = 2 * bjp + u
                    lo = max(0, (bj - (BJ - 4)) * P)
                    nc.tensor.matmul(
                        ps[:, u, lo:],
                        lhsT=ks[:, h, bj * P:(bj + 1) * P],
                        rhs=qs[:, h, ci * 512 + lo:(ci + 1) * 512],
                        start=True, stop=True,
                    )
                nc.scalar.activation(
                    out=pt[:, 2 * bjp:2 * bjp + 2, :], in_=ps,
                    func=mybir.ActivationFunctionType.Exp)
                if filled < fill_cap and op_state["pending"]:
                    emit_opchain()
                    filled += 1

        def av_unit(ci, h):
            """Strip masks + A@V + normalize + spill/transpose/fp8-split."""
            BJ = 4 * (ci + 1)
            pt = pt_tiles.pop((ci, h))
            # triangle masks: only the [rr*128, rr*128+128) strip of each
            # diagonal block is read by a chain that needs masking
            for rr in range(4):
                bj = BJ - 4 + rr
                csl = slice(rr * P, (rr + 1) * P)
                nc.gpsimd.tensor_mul(
                    out=pt[:, bj, csl], in0=pt[:, bj, csl],
                    in1=mask_sb[:, rr, csl])

            # A @ [V | 1/SA]: out rows are queries, col 128 holds denom/SA;
            # normalize (and fp8-prescale) on eviction. Chain ib only sums
            # blocks bj <= BJ-4+ib — later diagonal blocks are all-masked.
            att_h = att_pool.tile([P, 4, D], FP16, tag="atth",
                                  name=f"att_{ci}_{h}")
            for ib in range(4):
                nbj = BJ - 3 + ib
                avp = av_ps.tile([P, D + 4], F32, tag="avps",
                                 name=f"avps_{ci}_{h}_{ib}")
                isl = slice(ib * P, (ib + 1) * P)
                for bj in range(nbj):
                    nc.tensor.matmul(
                        avp[:, :D + 1],
                        lhsT=pt[:, bj, isl],
                        rhs=v_all[:, bj, h, :D + 1],
                        start=(bj == 0), stop=(bj == nbj - 1),
                    )
                rs = rs_pool.tile([P, 1], F32, tag="rs",
                                  name=f"rs_{ci}_{h}_{ib}")
                nc.vector.reciprocal_approx_fast(
                    out=rs, in_=avp[:, D:D + 1])
                nc.vector.tensor_scalar_mul(
                    out=att_h[:, ib, :], in0=avp[:, :D], scalar1=rs)

            # transpose + fp8 hi/lo split right away so the out-projection
            # chains for this chunk unlock early. Chunk 0 (inside the proj
            # phase, PSUM full) takes the DRAM spill + XBAR-transpose path;
            # later chunks transpose on the PE via the identity matrix and
            # quantize straight out of PSUM.
            nsl = slice(ci * 512, (ci + 1) * 512)
            if op_state["tr"] is None:
                nc.sync.dma_start(
                    out=attd[h, nsl, :].rearrange("(io p) d -> p io d", p=P),
                    in_=att_h)
                nc.sync.dma_start_transpose(
                    out=outT16[:, h, nsl], in_=attd[h, nsl, :])
                nc.gpsimd.tensor_copy(out=outTh[:, h, nsl],
                                      in_=outT16[:, h, nsl])
                nc.vector.tensor_sub(
                    out=outTl[:, h, nsl], in0=outT16[:, h, nsl],
                    in1=outTh[:, h, nsl])
            else:
                for ib in range(4):
                    tr = av_ps.tile([P, P], FP16, tag="avps",
                                    name=f"tr_{ci}_{h}_{ib}")
                    nc.tensor.matmul(tr, lhsT=att_h[:, ib, :], rhs=ident_sb,
                                     is_transpose=True, start=True, stop=True)
                    asl = slice(ci * 512 + ib * P, ci * 512 + (ib + 1) * P)
                    nc.vector.tensor_copy(out=outTh[:, h, asl], in_=tr)
                    nc.vector.tensor_sub(
                        out=outTl[:, h, asl], in0=tr, in1=outTh[:, h, asl])
            if h == HPC - 1:
                # this chunk's out-projection blocks are now computable
                dest = "pending" if ci == 0 else "fresh"
                for nb in range(ci * 4, ci * 4 + 4):
                    for ec in range(NCH):
                        op_state[dest].append((nb, ec))

        # ---------------- phase 1: projections (+ attention chunk 0) -------
        pt0_pool = tc.alloc_tile_pool(name="pt0_pool", bufs=2)

        with (
            tc.tile_pool(name="wpool", bufs=1) as wpool,
            tc.tile_pool(name="xpool", bufs=2) as xpool,
            tc.tile_pool(name="pj_ps", bufs=2, space="PSUM") as pj_ps,
        ):
            wqh_sb = wpool.tile([P, ET, DC], FP8)
            wql_sb = wpool.tile([P, ET, DC], FP8)
            wkh_sb = wpool.tile([P, ET, DC], FP8)
            wkl_sb = wpool.tile([P, ET, DC], FP8)
            wvh_sb = wpool.tile([P, ET, DC], FP8)
            wvl_sb = wpool.tile([P, ET, DC], FP8)
            x_tiles = [None] * NCH

            def load_x(nch, eng):
                th = xpool.tile([P, ET, 512], FP8, tag="xh",
                                name=f"xh_sb{nch}")
                tl = xpool.tile([P, ET, 512], FP8, tag="xl",
                                name=f"xl_sb{nch}")
                nsl = slice(nch * 512, (nch + 1) * 512)
                eng.dma_start(out=th, in_=xTh_r[:, :, nsl])
                eng.dma_start(out=tl, in_=xTl_r[:, :, nsl])
                x_tiles[nch] = (th, tl)

            # preload split across the two HWDGE queues so each stream
            # arrives before its first consuming chain:
            #   SP queue:  wq (fine-grained head), wv, mask, x2
            #   Act queue: x0 (fine-grained), wk, x1, wo, x3
            x0h = xpool.tile([P, ET, 512], FP8, tag="xh", name="xh_sb0")
            x0l = xpool.tile([P, ET, 512], FP8, tag="xl", name="xl_sb0")
            x_tiles[0] = (x0h, x0l)
            HF = ET // 2
            for a, b in ((0, 2), (2, 4), (4, 8), (8, 16)):
                gs = slice(a, b)
                nc.sync.dma_start(out=wqh_sb[:, gs, :], in_=wqTh_r[:, gs, :])
                nc.scalar.dma_start(out=x0h[:, gs, :], in_=xTh_r[:, gs, 0:512])
            for gs in (slice(0, HF), slice(HF, ET)):
                nc.sync.dma_start(out=wql_sb[:, gs, :], in_=wqTl_r[:, gs, :])
                nc.scalar.dma_start(out=x0l[:, gs, :], in_=xTl_r[:, gs, 0:512])
            for gs in (slice(0, HF), slice(HF, ET)):
                nc.scalar.dma_start(out=wkh_sb[:, gs, :], in_=wkTh_r[:, gs, :])
            nc.scalar.dma_start(out=wkl_sb, in_=wkTl_r)
            # wv split across both queues so it lands before x1/wo transfers
            nc.sync.dma_start(out=wvh_sb[:, :HF, :], in_=wvTh_r[:, :HF, :])
            nc.scalar.dma_start(out=wvh_sb[:, HF:, :], in_=wvTh_r[:, HF:, :])
            nc.sync.dma_start(out=wvl_sb[:, :HF, :], in_=wvTl_r[:, :HF, :])
            nc.scalar.dma_start(out=wvl_sb[:, HF:, :], in_=wvTl_r[:, HF:, :])
            nc.sync.dma_start(out=mask_sb, in_=maskin.ap())
            nc.sync.dma_start(out=ident_sb, in_=ident.ap())
            load_x(1, nc.scalar)
            for t in range(HPC):
                nc.scalar.dma_start(out=woh_sb[:, t, :], in_=woTh_r[:, t, :])
                nc.scalar.dma_start(out=wol_sb[:, t, :], in_=woTl_r[:, t, :])

            def proj_qk(nch, which):
                xh_sb, xl_sb = x_tiles[nch]
                nsl = slice(nch * 512, (nch + 1) * 512)
                wh_sb, wl_sb, dst, dsc = {
                    "q": (wqh_sb, wql_sb, qs, iq),
                    "k": (wkh_sb, wkl_sb, ks, ik),
                }[which]
                # psum[dq_tile 128, 512 n] = sum_e W[e, dq] x[e, n]
                for t in range(HPC):
                    tsl = slice(t * P, (t + 1) * P)
                    ps = pj_ps.tile([P, 512], F32, tag="pjps",
                                    name=f"pjps_{which}{nch}_{t}")
                    n_mm = 0
                    for wpass, xpass in ((wh_sb, xh_sb), (wh_sb, xl_sb),
                                         (wl_sb, xh_sb)):
                        for kp in range(ET // 2):
                            es = slice(2 * kp, 2 * kp + 2)
                            nc.tensor.matmul(
                                ps,
                                lhsT=wpass[:, es, tsl],
                                rhs=xpass[:, es, :],
                                start=(n_mm == 0),
                                stop=(n_mm == 3 * ET // 2 - 1),
                                perf_mode=DR,
                            )
                            n_mm += 1
                    if nch == NCH - 1 and t < 2:
                        nc.scalar.mul(dst[:, t, nsl], ps, dsc)
                    else:
                        nc.vector.tensor_scalar_mul(
                            out=dst[:, t, nsl], in0=ps, scalar1=dsc)

            def proj_v(nch):
                xh_sb, xl_sb = x_tiles[nch]
                # psum[n_block 128, 512 dv] = sum_e x[e, n] Wv[e, dv]
                for nb in range(4):
                    nbsl = slice(nb * P, (nb + 1) * P)
                    ps = pj_ps.tile([P, 512], F32, tag="pjps",
                                    name=f"pjps_v{nch}_{nb}")
                    n_mm = 0
                    for xpass, wpass in ((xh_sb, wvh_sb), (xh_sb, wvl_sb),
                                         (xl_sb, wvh_sb)):
                        for kp in range(ET // 2):
                            es = slice(2 * kp, 2 * kp + 2)
                            nc.tensor.matmul(
                                ps,
                                lhsT=xpass[:, es, nbsl],
                                rhs=wpass[:, es, :],
                                start=(n_mm == 0),
                                stop=(n_mm == 3 * ET // 2 - 1),
                                perf_mode=DR,
                            )
                            n_mm += 1
                    ps_h = ps.rearrange("p (h d) -> p h d", h=HPC)
                    vdst = v_all[:, nch * 4 + nb, :, :D]
                    if nch == NCH - 1 and nb < 2:
                        nc.scalar.mul(vdst, ps_h, iv)
                    else:
                        nc.vector.tensor_scalar_mul(
                            out=vdst, in0=ps_h, scalar1=iv)

            def proj_chunk(nch):
                proj_qk(nch, "q")
                proj_qk(nch, "k")
                proj_v(nch)
                if nch + 2 < NCH:
                    load_x(nch + 2, nc.sync if nch == 0 else nc.scalar)

            proj_chunk(0)
            proj_chunk(1)
            # attention chunk 0 interleaves with proj chunks 2-3 at head
            # granularity: each head's exp latency hides under a group of
            # four projection chains before its A@V runs
            qk_unit(0, 0, pt0_pool, "pt0")
            proj_qk(2, "q")
            av_unit(0, 0)
            qk_unit(0, 1, pt0_pool, "pt0")
            proj_qk(2, "k")
            av_unit(0, 1)
            qk_unit(0, 2, pt0_pool, "pt0")
            proj_v(2)
            av_unit(0, 2)
            qk_unit(0, 3, pt0_pool, "pt0")
            proj_qk(3, "q")
            av_unit(0, 3)
            proj_qk(3, "k")
            proj_v(3)

        pt0_pool.release()

        # ---------------- phase 2: attention chunks 1-3 + out-projection ---
        op_ps = tc.alloc_tile_pool(name="op_ps", bufs=2, space="PSUM")
        op_ev = tc.alloc_tile_pool(name="op_ev", bufs=6)
        op_state["pool"] = op_ps
        op_state["ev"] = op_ev
        op_state["tr"] = True

        pt_pool = tc.alloc_tile_pool(name="pt_pool", bufs=2)
        fill_caps = {1: 3, 2: 4, 3: 5}
        for ci in range(1, NCH):
            for h in range(HPC):
                qk_unit(ci, h, pt_pool, f"pt{ci}", fill_cap=fill_caps[ci])
                av_unit(ci, h)

        # drain the remaining out-projection chains (last chunk's blocks)
        op_state["pending"].extend(op_state["fresh"])
        op_state["fresh"] = []
        op_state["drain"] = True
        while op_state["pending"]:
            emit_opchain()

        pt_pool.release()
        op_ev.release()
        op_ps.release()
        for _pl in reversed(_longlived):
            _pl.release()

    nc.compile()
    return nc


def _po2(target_rms, arr):
    rms = float(np.sqrt(np.mean(arr.astype(np.float64) ** 2)))
    if rms == 0.0 or not math.isfinite(rms):
        return 1.0
    return 2.0 ** round(math.log2(target_rms / rms))


def _hilo(a32):
    hi = a32.astype(E4M3)
    lo = (a32 - hi.astype(np.float32)).astype(E4M3)
    return hi, lo


def make_in_maps(x, Wq, Wkv, Wout):
    x = np.asarray(x, dtype=np.float32)
    Wq = np.asarray(Wq, dtype=np.float32)
    Wkv = np.asarray(Wkv, dtype=np.float32)
    Wout = np.asarray(Wout, dtype=np.float32)
    scale = np.float32(D ** -0.5)

    sx = _po2(3.0, x)
    sq = _po2(3.0, Wq * scale)
    sk = _po2(3.0, Wkv[:E])
    sv = _po2(3.0, Wkv[E:])
    so = _po2(3.0, Wout)
    scales = (sx, sq, sk, sv, so)

    # causal masks for the 4 diagonal offsets
    jj = np.arange(P)[:, None]
    ii = np.arange(512)[None, :]
    mask = np.zeros((P, 4, 512), dtype=np.float16)
    for rr in range(4):
        mask[:, rr, :] = (ii >= jj + rr * P).astype(np.float16)

    xT = [_hilo(np.ascontiguousarray(x[b].T) * sx) for b in range(B)]
    in_maps = []
    for c in range(NCORES):
        b, hg = divmod(c, 4)
        sl = slice(hg * DC, (hg + 1) * DC)
        wq_h, wq_l = _hilo(np.ascontiguousarray(Wq[sl, :].T) * (scale * sq))
        wk_h, wk_l = _hilo(np.ascontiguousarray(Wkv[sl, :].T) * sk)
        wv_h, wv_l = _hilo(
            np.ascontiguousarray(Wkv[E + sl.start:E + sl.stop, :].T) * sv)
        wo_h, wo_l = _hilo(np.ascontiguousarray(Wout[:, sl].T) * so)
        in_maps.append({
            "xTh": xT[b][0], "xTl": xT[b][1],
            "wqTh": wq_h, "wqTl": wq_l,
            "wkTh": wk_h, "wkTl": wk_l,
            "wvTh": wv_h, "wvTl": wv_l,
            "woTh": wo_h, "woTl": wo_l,
            "maskin": mask,
            "ident": np.eye(P, dtype=np.float16),
        })
    return in_maps, scales


_NC_CACHE = {}
_LAST_SCALES = [None]


def _get_nc(scales=None):
    if scales is None:
        scales = _LAST_SCALES[0]
        assert scales is not None, "call make_in_maps first"
    if scales not in _NC_CACHE:
        _NC_CACHE[scales] = build_nc(*scales)
    _LAST_SCALES[0] = scales
    return _NC_CACHE[scales]


def _run(in_maps, scales):
    nc = _get_nc(scales)
    return run_bass_kernel_spmd(nc, in_maps, core_ids=list(range(NCORES)))


def kernel(x, Wq, Wkv, Wout):
    in_maps, scales = make_in_maps(x, Wq, Wkv, Wout)
    res = _run(in_maps, scales)
    out = np.zeros((B, N, E), dtype=np.float32)
    for c in range(NCORES):
        out[c // 4] += res.results[c]["out"].astype(np.float32)
    return out


if __name__ == "__main__":
    t0 = time.time()
    rng = np.random.default_rng(0)
    ins = {
        "x": rng.standard_normal((B, N, E), dtype=np.float32),
        "Wq": rng.standard_normal((E, E), dtype=np.float32) * 0.02,
        "Wkv": rng.standard_normal((2 * E, E), dtype=np.float32) * 0.02,
        "Wout": rng.standard_normal((E, E), dtype=np.float32) * 0.02,
    }
    im, sc = make_in_maps(**ins)
    _get_nc(sc)
    print(f"build+compile: {time.time() - t0:.1f}s")
